# revision 1
# baseline (speedup 1.0000x reference)
"""Trainium2 Bass kernel for ContextQueryAttention (trilinear attention).

Math (per batch b; C:[D,N], Q:[D,M], W0:[3D], b0:[1]):
    Ct = C.T, Qt = Q.T
    S[n,m] = Ct@w_c [n] + Qt@w_q [m] + sum_d Ct[n,d]*w_qc[d]*Qt[m,d] + b0
    S_row = softmax_m(S), S_col = softmax_n(S)
    A  = S_row @ Qt                       # (N, D)
    Bt = (S_row @ S_col.T) @ Ct           # (N, D)

Key algebraic restructurings used here:
  * Bt = S_row @ (S_col.T @ Ct)  -- drops the N x N intermediate entirely
    (805 MFLOP/batch -> 134 MFLOP/batch).
  * softmax_m is invariant to per-row constants, softmax_n to per-column
    constants, so the row path only needs the q-score bias and the col path
    only the c-score bias; b0 cancels everywhere.
  * Input magnitudes are O(5), so exp() needs no max-subtraction.
  * Softmax denominators come for free as extra all-ones columns fused
    into the consuming matmuls; normalization folds into per-partition
    scalar multiplies after the matmuls.
  * All matmuls run in float32r (full-rate fp32); moving free sizes kept
    even (hw requirement) by duplicating the fused score/ones columns.

Sharding: data-parallel over batch, 8 batches per core on 8 cores.
"""

import numpy as np

import concourse.bass as bass
import concourse.bacc as bacc
import concourse.tile as tile
from concourse import mybir
from concourse.bass_utils import run_bass_kernel_spmd
from concourse.masks import make_identity

F32 = mybir.dt.float32
F32R = mybir.dt.float32r

# Problem shape (hardcoded per spec)
B, D, N, M = 64, 128, 1024, 256
NCORES = 8
BPC = B // NCORES  # batches per core
NK = N // 128      # context chunks (8)
MJ = M // 128      # query chunks (2)


def build_kernel(bpc: int = BPC, repeats: int = 1) -> bass.Bass:
    nc = bacc.Bacc("TRN2", target_bir_lowering=False, debug=False)

    C8 = nc.dram_tensor("C", [bpc, D, N], F32, kind="ExternalInput").ap()
    Q8 = nc.dram_tensor("Q", [bpc, D, M], F32, kind="ExternalInput").ap()
    W0 = nc.dram_tensor("W0", [3 * D], F32, kind="ExternalInput").ap()
    A8 = nc.dram_tensor("A", [bpc, N, D], F32, kind="ExternalOutput").ap()
    B8 = nc.dram_tensor("Bt", [bpc, N, D], F32, kind="ExternalOutput").ap()

    with tile.TileContext(nc) as tc:
        with (
            tc.tile_pool(name="singles", bufs=1) as singles,
            tc.tile_pool(name="inp", bufs=2) as pool_in,
            tc.tile_pool(name="scaled", bufs=2) as pool_sc,
            tc.tile_pool(name="ct", bufs=2) as pool_ct,
            tc.tile_pool(name="e", bufs=2) as pool_e,
            tc.tile_pool(name="qtg", bufs=2) as pool_qtg,
            tc.tile_pool(name="small", bufs=2) as pool_sm,
            tc.tile_pool(name="out", bufs=3) as pool_out,
            tc.tile_pool(name="pp_t", bufs=2, space="PSUM") as pp_t,
            tc.tile_pool(name="pp_x", bufs=2, space="PSUM") as pp_x,
            tc.tile_pool(name="pp_xt", bufs=2, space="PSUM") as pp_xt,
        ):
            # --- constants ---
            # wvec cols: w_q, w_q, w_c, w_c, w_qc  (score columns doubled so
            # fused matmul moving sizes stay even, as float32r requires)
            wvec = singles.tile([D, 5], F32)
            for i, s in enumerate((0, 0, 1, 1, 2)):
                nc.sync.dma_start(
                    out=wvec[:, i : i + 1],
                    in_=W0[s * D : (s + 1) * D].rearrange("(p o) -> p o", o=1),
                )
            w_qc = wvec[:, 4:5]
            ones2 = singles.tile([128, 2], F32)
            nc.vector.memset(ones2, 1.0)
            ident_f32 = singles.tile([128, 128], F32)
            make_identity(nc, ident_f32)
            ident = singles.tile([128, 128], F32R)
            nc.vector.tensor_copy(out=ident, in_=ident_f32)

            import contextlib

            rep_ctx = (
                tc.For_i(
                    0,
                    repeats,
                    1,
                    hint_engines=(
                        mybir.EngineType.PE,
                        mybir.EngineType.DVE,
                        mybir.EngineType.Activation,
                        mybir.EngineType.SP,
                    ),
                )
                if repeats > 1
                else contextlib.nullcontext()
            )
            with rep_ctx:
              for b in range(bpc):
                cb = pool_in.tile([D, N], F32R, tag="cb")
                qb = pool_in.tile([D, M], F32R, tag="qb")
                nc.sync.dma_start(out=cb, in_=C8[b].bitcast(F32R))
                nc.sync.dma_start(out=qb, in_=Q8[b].bitcast(F32R))

                # scaled inputs with fused (doubled) score columns
                # cswq = [C * w_qc | w_q w_q]  -> rhs for X^T and QS matmuls
                cswq = pool_sc.tile([D, N + 2], F32R, tag="cswq")
                nc.vector.tensor_scalar_mul(out=cswq[:, 0:N], in0=cb, scalar1=w_qc)
                nc.vector.tensor_copy(out=cswq[:, N : N + 2], in_=wvec[:, 0:2])
                # qswc = [Q * w_qc | w_c w_c]  -> rhs for X matmuls
                qswc = pool_sc.tile([D, M + 2], F32R, tag="qswc")
                nc.vector.tensor_scalar_mul(out=qswc[:, 0:M], in0=qb, scalar1=w_qc)
                nc.vector.tensor_copy(out=qswc[:, M : M + 2], in_=wvec[:, 2:4])

                # --- transposes: ct_k = [Ct_k | 1 1], qtg_j = [Qt_j | 1 1 | G_j]
                ct = pool_ct.tile([128, NK, D + 2], F32R, tag="ct")
                for k in range(NK):
                    pt = pp_t.tile([128, 128], F32R, tag="pt")
                    nc.tensor.transpose(pt, cb[:, k * 128 : (k + 1) * 128], ident)
                    nc.vector.tensor_copy(out=ct[:, k, 0:D], in_=pt)
                    nc.vector.tensor_copy(out=ct[:, k, D : D + 2], in_=ones2)

                qtg = pool_qtg.tile([128, MJ, 2 * D + 2], F32R, tag="qtg")
                for j in range(MJ):
                    pt = pp_t.tile([128, 128], F32R, tag="pt")
                    nc.tensor.transpose(pt, qb[:, j * 128 : (j + 1) * 128], ident)
                    nc.vector.tensor_copy(out=qtg[:, j, 0:D], in_=pt)
                    nc.vector.tensor_copy(out=qtg[:, j, D : D + 2], in_=ones2)

                # --- X [n,m] chunks + col-softmax numerator E ---
                e_col = pool_e.tile([128, NK, M], F32R, tag="e_col")
                for k in range(NK):
                    px = pp_x.tile([128, M + 2], F32, tag="px")
                    nc.tensor.matmul(
                        px, cb[:, k * 128 : (k + 1) * 128], qswc, start=True, stop=True
                    )
                    cs_k = pool_sm.tile([128, 1], F32, tag=f"cs{k}")
                    nc.vector.tensor_copy(out=cs_k, in_=px[:, M : M + 1])
                    nc.scalar.activation(
                        out=e_col[:, k, :],
                        in_=px[:, 0:M],
                        func=mybir.ActivationFunctionType.Exp,
                        bias=cs_k,
                    )

                # --- X^T [m,n] chunks + row-softmax numerator E' ---
                e_row = pool_e.tile([128, MJ, N], F32R, tag="e_row")
                for j in range(MJ):
                    qbj = qb[:, j * 128 : (j + 1) * 128]
                    pxt = pp_xt.tile([128, N], F32, tag="pxt")
                    for h in range(N // 512):
                        nc.tensor.matmul(
                            pxt[:, h * 512 : (h + 1) * 512],
                            qbj,
                            cswq[:, h * 512 : (h + 1) * 512],
                            start=True,
                            stop=True,
                        )
                    pq = pp_t.tile([128, 128], F32, tag="pt")
                    nc.tensor.matmul(
                        pq[:, 0:2], qbj, cswq[:, N : N + 2], start=True, stop=True
                    )
                    qs_j = pool_sm.tile([128, 1], F32, tag=f"qs{j}")
                    nc.vector.tensor_copy(out=qs_j, in_=pq[:, 0:1])
                    nc.scalar.activation(
                        out=e_row[:, j, :],
                        in_=pxt,
                        func=mybir.ActivationFunctionType.Exp,
                        bias=qs_j,
                    )

                # --- col path: G_j = normalize(E^T @ [Ct|1 1]) ---
                for j in range(MJ):
                    pg = pp_t.tile([128, D + 2], F32, tag="pt")
                    for k in range(NK):
                        nc.tensor.matmul(
                            pg,
                            e_col[:, k, j * 128 : (j + 1) * 128],
                            ct[:, k, :],
                            start=(k == 0),
                            stop=(k == NK - 1),
                        )
                    rcol = pool_sm.tile([128, 1], F32, tag=f"rcol{j}")
                    nc.vector.reciprocal(out=rcol, in_=pg[:, D : D + 1])
                    nc.vector.tensor_scalar_mul(
                        out=qtg[:, j, D + 2 : 2 * D + 2], in0=pg[:, 0:D], scalar1=rcol
                    )

                # --- row path: [A | rowsum rowsum | Bt] = E'^T @ [Qt|1 1|G] ---
                for k in range(NK):
                    pab = pp_x.tile([128, 2 * D + 2], F32, tag="px")
                    for j in range(MJ):
                        nc.tensor.matmul(
                            pab,
                            e_row[:, j, k * 128 : (k + 1) * 128],
                            qtg[:, j, :],
                            start=(j == 0),
                            stop=(j == MJ - 1),
                        )
                    rrow = pool_sm.tile([128, 1], F32, tag=f"rrow{k}")
                    nc.vector.reciprocal(out=rrow, in_=pab[:, D : D + 1])
                    oab = pool_out.tile([128, 2 * D], F32, tag="oab")
                    nc.vector.tensor_scalar_mul(
                        out=oab[:, 0:D], in0=pab[:, 0:D], scalar1=rrow
                    )
                    nc.vector.tensor_scalar_mul(
                        out=oab[:, D : 2 * D], in0=pab[:, D + 2 : 2 * D + 2], scalar1=rrow
                    )
                    nc.sync.dma_start(
                        out=A8[b, k * 128 : (k + 1) * 128, :], in_=oab[:, 0:D]
                    )
                    nc.sync.dma_start(
                        out=B8[b, k * 128 : (k + 1) * 128, :], in_=oab[:, D : 2 * D]
                    )
    nc.finalize()
    return nc


_NC_CACHE = None


def kernel(C, Q, W0, b0, _trace=False):
    global _NC_CACHE
    if _NC_CACHE is None:
        _NC_CACHE = build_kernel()
    nc = _NC_CACHE

    C = np.ascontiguousarray(np.asarray(C, dtype=np.float32))
    Q = np.ascontiguousarray(np.asarray(Q, dtype=np.float32))
    W0 = np.ascontiguousarray(np.asarray(W0, dtype=np.float32))

    in_maps = [
        {
            "C": C[i * BPC : (i + 1) * BPC],
            "Q": Q[i * BPC : (i + 1) * BPC],
            "W0": W0,
        }
        for i in range(NCORES)
    ]
    res = run_bass_kernel_spmd(nc, in_maps, core_ids=list(range(NCORES)))
    A = np.concatenate([res.results[i]["A"] for i in range(NCORES)], axis=0)
    Bt = np.concatenate([res.results[i]["Bt"] for i in range(NCORES)], axis=0)
    return (A, Bt)



# revision 3
# speedup vs baseline: 2.9463x; 2.9463x over previous
"""Trainium2 Bass kernel for ContextQueryAttention (trilinear attention).

Math (per batch b; C:[D,N], Q:[D,M], W0:[3D], b0:[1]):
    Ct = C.T, Qt = Q.T
    S[n,m] = Ct@w_c [n] + Qt@w_q [m] + sum_d Ct[n,d]*w_qc[d]*Qt[m,d] + b0
    S_row = softmax_m(S), S_col = softmax_n(S)
    A  = S_row @ Qt                       # (N, D)
    Bt = (S_row @ S_col.T) @ Ct           # (N, D)

Key algebraic restructurings used here:
  * Bt = S_row @ (S_col.T @ Ct)  -- drops the N x N intermediate entirely
    (805 MFLOP/batch -> 134 MFLOP/batch).
  * softmax_m is invariant to per-row constants, softmax_n to per-column
    constants, so the row path only needs the q-score bias and the col path
    only the c-score bias; b0 cancels everywhere.
  * Input magnitudes are O(5), so exp() needs no max-subtraction.
  * Softmax denominators come for free as extra all-ones columns fused
    into the consuming matmuls; normalization folds into per-partition
    scalar multiplies after the matmuls.
  * All matmuls run in float32r (full-rate fp32); moving free sizes kept
    even (hw requirement) by duplicating the fused score/ones columns.

Sharding: all 64 batches on ONE core. Multi-device dispatch through the
axon-tunneled PJRT path costs ~1.5ms per extra-device group per call
(8-core: ~12ms, 2-core: ~8ms, 1-core: ~3ms) while the actual compute is
~0.4ms, so a single core minimizes end-to-end per-call time.
"""

import numpy as np

import concourse.bass as bass
import concourse.bacc as bacc
import concourse.tile as tile
from concourse import mybir
from concourse.bass_utils import run_bass_kernel_spmd
from concourse.masks import make_identity

F32 = mybir.dt.float32
F32R = mybir.dt.float32r

# Problem shape (hardcoded per spec)
B, D, N, M = 64, 128, 1024, 256
NCORES = 1
BPC = B // NCORES  # batches per core
NK = N // 128      # context chunks (8)
MJ = M // 128      # query chunks (2)


def build_kernel(bpc: int = BPC, repeats: int = 1) -> bass.Bass:
    nc = bacc.Bacc("TRN2", target_bir_lowering=False, debug=False)

    C8 = nc.dram_tensor("C", [bpc, D, N], F32, kind="ExternalInput").ap()
    Q8 = nc.dram_tensor("Q", [bpc, D, M], F32, kind="ExternalInput").ap()
    W0 = nc.dram_tensor("W0", [3 * D], F32, kind="ExternalInput").ap()
    A8 = nc.dram_tensor("A", [bpc, N, D], F32, kind="ExternalOutput").ap()
    B8 = nc.dram_tensor("Bt", [bpc, N, D], F32, kind="ExternalOutput").ap()

    with tile.TileContext(nc) as tc:
        with (
            tc.tile_pool(name="singles", bufs=1) as singles,
            tc.tile_pool(name="inp", bufs=2) as pool_in,
            tc.tile_pool(name="scaled", bufs=2) as pool_sc,
            tc.tile_pool(name="ct", bufs=2) as pool_ct,
            tc.tile_pool(name="e", bufs=2) as pool_e,
            tc.tile_pool(name="qtg", bufs=2) as pool_qtg,
            tc.tile_pool(name="small", bufs=2) as pool_sm,
            tc.tile_pool(name="out", bufs=3) as pool_out,
            tc.tile_pool(name="pp_t", bufs=2, space="PSUM") as pp_t,
            tc.tile_pool(name="pp_x", bufs=2, space="PSUM") as pp_x,
            tc.tile_pool(name="pp_xt", bufs=2, space="PSUM") as pp_xt,
        ):
            # --- constants ---
            # wvec cols: w_q, w_q, w_c, w_c, w_qc  (score columns doubled so
            # fused matmul moving sizes stay even, as float32r requires)
            wvec = singles.tile([D, 5], F32)
            for i, s in enumerate((0, 0, 1, 1, 2)):
                nc.sync.dma_start(
                    out=wvec[:, i : i + 1],
                    in_=W0[s * D : (s + 1) * D].rearrange("(p o) -> p o", o=1),
                )
            w_qc = wvec[:, 4:5]
            ones2 = singles.tile([128, 2], F32)
            nc.vector.memset(ones2, 1.0)
            ident_f32 = singles.tile([128, 128], F32)
            make_identity(nc, ident_f32)
            ident = singles.tile([128, 128], F32R)
            nc.vector.tensor_copy(out=ident, in_=ident_f32)

            import contextlib

            rep_ctx = (
                tc.For_i(
                    0,
                    repeats,
                    1,
                    hint_engines=(
                        mybir.EngineType.PE,
                        mybir.EngineType.DVE,
                        mybir.EngineType.Activation,
                        mybir.EngineType.SP,
                    ),
                )
                if repeats > 1
                else contextlib.nullcontext()
            )
            with rep_ctx:
              for b in range(bpc):
                cb = pool_in.tile([D, N], F32R, tag="cb")
                qb = pool_in.tile([D, M], F32R, tag="qb")
                nc.sync.dma_start(out=cb, in_=C8[b].bitcast(F32R))
                nc.sync.dma_start(out=qb, in_=Q8[b].bitcast(F32R))

                # scaled inputs with fused (doubled) score columns
                # cswq = [C * w_qc | w_q w_q]  -> rhs for X^T and QS matmuls
                cswq = pool_sc.tile([D, N + 2], F32R, tag="cswq")
                nc.vector.tensor_scalar_mul(out=cswq[:, 0:N], in0=cb, scalar1=w_qc)
                nc.vector.tensor_copy(out=cswq[:, N : N + 2], in_=wvec[:, 0:2])
                # qswc = [Q * w_qc | w_c w_c]  -> rhs for X matmuls
                qswc = pool_sc.tile([D, M + 2], F32R, tag="qswc")
                nc.vector.tensor_scalar_mul(out=qswc[:, 0:M], in0=qb, scalar1=w_qc)
                nc.vector.tensor_copy(out=qswc[:, M : M + 2], in_=wvec[:, 2:4])

                # --- transposes: ct_k = [Ct_k | 1 1], qtg_j = [Qt_j | 1 1 | G_j]
                ct = pool_ct.tile([128, NK, D + 2], F32R, tag="ct")
                for k in range(NK):
                    pt = pp_t.tile([128, 128], F32R, tag="pt")
                    nc.tensor.transpose(pt, cb[:, k * 128 : (k + 1) * 128], ident)
                    nc.vector.tensor_copy(out=ct[:, k, 0:D], in_=pt)
                    nc.vector.tensor_copy(out=ct[:, k, D : D + 2], in_=ones2)

                qtg = pool_qtg.tile([128, MJ, 2 * D + 2], F32R, tag="qtg")
                for j in range(MJ):
                    pt = pp_t.tile([128, 128], F32R, tag="pt")
                    nc.tensor.transpose(pt, qb[:, j * 128 : (j + 1) * 128], ident)
                    nc.vector.tensor_copy(out=qtg[:, j, 0:D], in_=pt)
                    nc.vector.tensor_copy(out=qtg[:, j, D : D + 2], in_=ones2)

                # --- X [n,m] chunks + col-softmax numerator E ---
                e_col = pool_e.tile([128, NK, M], F32R, tag="e_col")
                for k in range(NK):
                    px = pp_x.tile([128, M + 2], F32, tag="px")
                    nc.tensor.matmul(
                        px, cb[:, k * 128 : (k + 1) * 128], qswc, start=True, stop=True
                    )
                    cs_k = pool_sm.tile([128, 1], F32, tag=f"cs{k}")
                    nc.vector.tensor_copy(out=cs_k, in_=px[:, M : M + 1])
                    nc.scalar.activation(
                        out=e_col[:, k, :],
                        in_=px[:, 0:M],
                        func=mybir.ActivationFunctionType.Exp,
                        bias=cs_k,
                    )

                # --- X^T [m,n] chunks + row-softmax numerator E' ---
                e_row = pool_e.tile([128, MJ, N], F32R, tag="e_row")
                for j in range(MJ):
                    qbj = qb[:, j * 128 : (j + 1) * 128]
                    pxt = pp_xt.tile([128, N], F32, tag="pxt")
                    for h in range(N // 512):
                        nc.tensor.matmul(
                            pxt[:, h * 512 : (h + 1) * 512],
                            qbj,
                            cswq[:, h * 512 : (h + 1) * 512],
                            start=True,
                            stop=True,
                        )
                    pq = pp_t.tile([128, 128], F32, tag="pt")
                    nc.tensor.matmul(
                        pq[:, 0:2], qbj, cswq[:, N : N + 2], start=True, stop=True
                    )
                    qs_j = pool_sm.tile([128, 1], F32, tag=f"qs{j}")
                    nc.vector.tensor_copy(out=qs_j, in_=pq[:, 0:1])
                    nc.scalar.activation(
                        out=e_row[:, j, :],
                        in_=pxt,
                        func=mybir.ActivationFunctionType.Exp,
                        bias=qs_j,
                    )

                # --- col path: G_j = normalize(E^T @ [Ct|1 1]) ---
                for j in range(MJ):
                    pg = pp_t.tile([128, D + 2], F32, tag="pt")
                    for k in range(NK):
                        nc.tensor.matmul(
                            pg,
                            e_col[:, k, j * 128 : (j + 1) * 128],
                            ct[:, k, :],
                            start=(k == 0),
                            stop=(k == NK - 1),
                        )
                    rcol = pool_sm.tile([128, 1], F32, tag=f"rcol{j}")
                    nc.vector.reciprocal(out=rcol, in_=pg[:, D : D + 1])
                    nc.vector.tensor_scalar_mul(
                        out=qtg[:, j, D + 2 : 2 * D + 2], in0=pg[:, 0:D], scalar1=rcol
                    )

                # --- row path: [A | rowsum rowsum | Bt] = E'^T @ [Qt|1 1|G] ---
                for k in range(NK):
                    pab = pp_x.tile([128, 2 * D + 2], F32, tag="px")
                    for j in range(MJ):
                        nc.tensor.matmul(
                            pab,
                            e_row[:, j, k * 128 : (k + 1) * 128],
                            qtg[:, j, :],
                            start=(j == 0),
                            stop=(j == MJ - 1),
                        )
                    rrow = pool_sm.tile([128, 1], F32, tag=f"rrow{k}")
                    nc.vector.reciprocal(out=rrow, in_=pab[:, D : D + 1])
                    oab = pool_out.tile([128, 2 * D], F32, tag="oab")
                    nc.vector.tensor_scalar_mul(
                        out=oab[:, 0:D], in0=pab[:, 0:D], scalar1=rrow
                    )
                    nc.vector.tensor_scalar_mul(
                        out=oab[:, D : 2 * D], in0=pab[:, D + 2 : 2 * D + 2], scalar1=rrow
                    )
                    nc.sync.dma_start(
                        out=A8[b, k * 128 : (k + 1) * 128, :], in_=oab[:, 0:D]
                    )
                    nc.sync.dma_start(
                        out=B8[b, k * 128 : (k + 1) * 128, :], in_=oab[:, D : 2 * D]
                    )
    nc.finalize()
    return nc


_NC_CACHE = None


def kernel(C, Q, W0, b0, _trace=False):
    global _NC_CACHE
    if _NC_CACHE is None:
        _NC_CACHE = build_kernel()
    nc = _NC_CACHE

    C = np.ascontiguousarray(np.asarray(C, dtype=np.float32))
    Q = np.ascontiguousarray(np.asarray(Q, dtype=np.float32))
    W0 = np.ascontiguousarray(np.asarray(W0, dtype=np.float32))

    in_maps = [
        {
            "C": C[i * BPC : (i + 1) * BPC],
            "Q": Q[i * BPC : (i + 1) * BPC],
            "W0": W0,
        }
        for i in range(NCORES)
    ]
    res = run_bass_kernel_spmd(nc, in_maps, core_ids=list(range(NCORES)))
    A = np.concatenate([res.results[i]["A"] for i in range(NCORES)], axis=0)
    Bt = np.concatenate([res.results[i]["Bt"] for i in range(NCORES)], axis=0)
    return (A, Bt)



# revision 4
# speedup vs baseline: 3.2902x; 1.1167x over previous
"""Trainium2 Bass kernel for ContextQueryAttention (trilinear attention).

Math (per batch b; C:[D,N], Q:[D,M], W0:[3D], b0:[1]):
    Ct = C.T, Qt = Q.T
    S[n,m] = Ct@w_c [n] + Qt@w_q [m] + sum_d Ct[n,d]*w_qc[d]*Qt[m,d] + b0
    S_row = softmax_m(S), S_col = softmax_n(S)
    A  = S_row @ Qt                       # (N, D)
    Bt = (S_row @ S_col.T) @ Ct           # (N, D)

Key algebraic restructurings used here:
  * Bt = S_row @ (S_col.T @ Ct)  -- drops the N x N intermediate entirely
    (805 MFLOP/batch -> 134 MFLOP/batch).
  * softmax_m is invariant to per-row constants, softmax_n to per-column
    constants, so the row path only needs the q-score bias and the col path
    only the c-score bias; b0 cancels everywhere.
  * Input magnitudes are O(5), so exp() needs no max-subtraction.
  * Softmax denominators come for free as extra all-ones columns fused
    into the consuming matmuls; normalization folds into per-partition
    scalar multiplies after the matmuls.
  * All matmuls run in float32r (full-rate fp32); moving free sizes kept
    even (hw requirement) by duplicating the fused score/ones columns.

Distribution: ALL 64 batches on ONE core, looped with a hardware For_i.
Rationale (measured, axon-tunneled PJRT path):
  * Per-call dispatch overhead dominates wall time; it scales with the
    number of devices (8-core ~12ms, 2-core ~8ms, 1-core ~3ms per call)
    while the actual device compute is <1ms and fully hidden behind the
    dispatch pipeline. One core minimizes the metric.
  * The per-call transport cost also scales with the NEFF instruction
    stream size (~87ns/instruction); a hardware For_i over batches keeps
    the stream at ~200 instructions instead of ~9600 fully unrolled.
"""

import numpy as np

import concourse.bass as bass
import concourse.bacc as bacc
import concourse.tile as tile
from concourse import mybir
from concourse.bass import ts
from concourse.bass_utils import run_bass_kernel_spmd
from concourse.masks import make_identity

F32 = mybir.dt.float32
F32R = mybir.dt.float32r

# Problem shape (hardcoded per spec)
B, D, N, M = 64, 128, 1024, 256
NCORES = 1
BPC = B // NCORES  # batches per core
NK = N // 128      # context chunks (8)
MJ = M // 128      # query chunks (2)


def build_kernel(bpc: int = BPC, dynamic: bool = True) -> bass.Bass:
    nc = bacc.Bacc("TRN2", target_bir_lowering=False, debug=False)

    C8 = nc.dram_tensor("C", [bpc, D, N], F32, kind="ExternalInput").ap()
    Q8 = nc.dram_tensor("Q", [bpc, D, M], F32, kind="ExternalInput").ap()
    W0 = nc.dram_tensor("W0", [3 * D], F32, kind="ExternalInput").ap()
    A8 = nc.dram_tensor("A", [bpc, N, D], F32, kind="ExternalOutput").ap()
    B8 = nc.dram_tensor("Bt", [bpc, N, D], F32, kind="ExternalOutput").ap()

    # flat row views for dynamic (runtime-index) batch addressing
    Cf = C8.rearrange("b d n -> (b d) n")
    Qf = Q8.rearrange("b d n -> (b d) n")
    Af = A8.rearrange("b n d -> (b n) d")
    Bf = B8.rearrange("b n d -> (b n) d")

    with tile.TileContext(nc) as tc:
        with (
            tc.tile_pool(name="singles", bufs=1) as singles,
            tc.tile_pool(name="inp", bufs=2) as pool_in,
            tc.tile_pool(name="scaled", bufs=2) as pool_sc,
            tc.tile_pool(name="ct", bufs=2) as pool_ct,
            tc.tile_pool(name="e", bufs=2) as pool_e,
            tc.tile_pool(name="qtg", bufs=2) as pool_qtg,
            tc.tile_pool(name="small", bufs=2) as pool_sm,
            tc.tile_pool(name="out", bufs=3) as pool_out,
            tc.tile_pool(name="pp_t", bufs=2, space="PSUM") as pp_t,
            tc.tile_pool(name="pp_x", bufs=2, space="PSUM") as pp_x,
            tc.tile_pool(name="pp_xt", bufs=2, space="PSUM") as pp_xt,
        ):
            # --- constants ---
            # wvec cols: w_q, w_q, w_c, w_c, w_qc  (score columns doubled so
            # fused matmul moving sizes stay even, as float32r requires)
            wvec = singles.tile([D, 5], F32)
            for i, s in enumerate((0, 0, 1, 1, 2)):
                nc.sync.dma_start(
                    out=wvec[:, i : i + 1],
                    in_=W0[s * D : (s + 1) * D].rearrange("(p o) -> p o", o=1),
                )
            w_qc = wvec[:, 4:5]
            ones2 = singles.tile([128, 2], F32)
            nc.vector.memset(ones2, 1.0)
            ident_f32 = singles.tile([128, 128], F32)
            make_identity(nc, ident_f32)
            ident = singles.tile([128, 128], F32R)
            nc.vector.tensor_copy(out=ident, in_=ident_f32)

            import contextlib

            loop_ctx = tc.For_i(0, bpc, 1) if dynamic else contextlib.nullcontext(0)
            with loop_ctx as bdyn:
              for bstat in range(1 if dynamic else bpc):
                b = bdyn if dynamic else bstat
                cb = pool_in.tile([D, N], F32R, tag="cb")
                qb = pool_in.tile([D, M], F32R, tag="qb")
                nc.sync.dma_start(out=cb, in_=Cf[ts(b, D), :].bitcast(F32R))
                nc.sync.dma_start(out=qb, in_=Qf[ts(b, D), :].bitcast(F32R))

                # scaled inputs with fused (doubled) score columns
                # cswq = [C * w_qc | w_q w_q]  -> rhs for X^T and QS matmuls
                cswq = pool_sc.tile([D, N + 2], F32R, tag="cswq")
                nc.vector.tensor_scalar_mul(out=cswq[:, 0:N], in0=cb, scalar1=w_qc)
                nc.vector.tensor_copy(out=cswq[:, N : N + 2], in_=wvec[:, 0:2])
                # qswc = [Q * w_qc | w_c w_c]  -> rhs for X matmuls
                qswc = pool_sc.tile([D, M + 2], F32R, tag="qswc")
                nc.vector.tensor_scalar_mul(out=qswc[:, 0:M], in0=qb, scalar1=w_qc)
                nc.vector.tensor_copy(out=qswc[:, M : M + 2], in_=wvec[:, 2:4])

                # --- transposes: ct_k = [Ct_k | 1 1], qtg_j = [Qt_j | 1 1 | G_j]
                ct = pool_ct.tile([128, NK, D + 2], F32R, tag="ct")
                for k in range(NK):
                    pt = pp_t.tile([128, 128], F32R, tag="pt")
                    nc.tensor.transpose(pt, cb[:, k * 128 : (k + 1) * 128], ident)
                    nc.vector.tensor_copy(out=ct[:, k, 0:D], in_=pt)
                    nc.vector.tensor_copy(out=ct[:, k, D : D + 2], in_=ones2)

                qtg = pool_qtg.tile([128, MJ, 2 * D + 2], F32R, tag="qtg")
                for j in range(MJ):
                    pt = pp_t.tile([128, 128], F32R, tag="pt")
                    nc.tensor.transpose(pt, qb[:, j * 128 : (j + 1) * 128], ident)
                    nc.vector.tensor_copy(out=qtg[:, j, 0:D], in_=pt)
                    nc.vector.tensor_copy(out=qtg[:, j, D : D + 2], in_=ones2)

                # --- X [n,m] chunks + col-softmax numerator E ---
                e_col = pool_e.tile([128, NK, M], F32R, tag="e_col")
                for k in range(NK):
                    px = pp_x.tile([128, M + 2], F32, tag="px")
                    nc.tensor.matmul(
                        px, cb[:, k * 128 : (k + 1) * 128], qswc, start=True, stop=True
                    )
                    cs_k = pool_sm.tile([128, 1], F32, tag=f"cs{k}")
                    nc.vector.tensor_copy(out=cs_k, in_=px[:, M : M + 1])
                    nc.scalar.activation(
                        out=e_col[:, k, :],
                        in_=px[:, 0:M],
                        func=mybir.ActivationFunctionType.Exp,
                        bias=cs_k,
                    )

                # --- X^T [m,n] chunks + row-softmax numerator E' ---
                e_row = pool_e.tile([128, MJ, N], F32R, tag="e_row")
                for j in range(MJ):
                    qbj = qb[:, j * 128 : (j + 1) * 128]
                    pxt = pp_xt.tile([128, N], F32, tag="pxt")
                    for h in range(N // 512):
                        nc.tensor.matmul(
                            pxt[:, h * 512 : (h + 1) * 512],
                            qbj,
                            cswq[:, h * 512 : (h + 1) * 512],
                            start=True,
                            stop=True,
                        )
                    pq = pp_t.tile([128, 128], F32, tag="pt")
                    nc.tensor.matmul(
                        pq[:, 0:2], qbj, cswq[:, N : N + 2], start=True, stop=True
                    )
                    qs_j = pool_sm.tile([128, 1], F32, tag=f"qs{j}")
                    nc.vector.tensor_copy(out=qs_j, in_=pq[:, 0:1])
                    nc.scalar.activation(
                        out=e_row[:, j, :],
                        in_=pxt,
                        func=mybir.ActivationFunctionType.Exp,
                        bias=qs_j,
                    )

                # --- col path: G_j = normalize(E^T @ [Ct|1 1]) ---
                for j in range(MJ):
                    pg = pp_t.tile([128, D + 2], F32, tag="pt")
                    for k in range(NK):
                        nc.tensor.matmul(
                            pg,
                            e_col[:, k, j * 128 : (j + 1) * 128],
                            ct[:, k, :],
                            start=(k == 0),
                            stop=(k == NK - 1),
                        )
                    rcol = pool_sm.tile([128, 1], F32, tag=f"rcol{j}")
                    nc.vector.reciprocal(out=rcol, in_=pg[:, D : D + 1])
                    nc.vector.tensor_scalar_mul(
                        out=qtg[:, j, D + 2 : 2 * D + 2], in0=pg[:, 0:D], scalar1=rcol
                    )

                # --- row path: [A | rowsum rowsum | Bt] = E'^T @ [Qt|1 1|G] ---
                for k in range(NK):
                    pab = pp_x.tile([128, 2 * D + 2], F32, tag="px")
                    for j in range(MJ):
                        nc.tensor.matmul(
                            pab,
                            e_row[:, j, k * 128 : (k + 1) * 128],
                            qtg[:, j, :],
                            start=(j == 0),
                            stop=(j == MJ - 1),
                        )
                    rrow = pool_sm.tile([128, 1], F32, tag=f"rrow{k}")
                    nc.vector.reciprocal(out=rrow, in_=pab[:, D : D + 1])
                    oab = pool_out.tile([128, 2 * D], F32, tag="oab")
                    nc.vector.tensor_scalar_mul(
                        out=oab[:, 0:D], in0=pab[:, 0:D], scalar1=rrow
                    )
                    nc.vector.tensor_scalar_mul(
                        out=oab[:, D : 2 * D], in0=pab[:, D + 2 : 2 * D + 2], scalar1=rrow
                    )
                    nc.sync.dma_start(
                        out=Af[ts(b * NK + k, 128), :], in_=oab[:, 0:D]
                    )
                    nc.sync.dma_start(
                        out=Bf[ts(b * NK + k, 128), :], in_=oab[:, D : 2 * D]
                    )
    nc.finalize()
    return nc


_NC_CACHE = None


def kernel(C, Q, W0, b0, _trace=False):
    global _NC_CACHE
    if _NC_CACHE is None:
        _NC_CACHE = build_kernel()
    nc = _NC_CACHE

    C = np.ascontiguousarray(np.asarray(C, dtype=np.float32))
    Q = np.ascontiguousarray(np.asarray(Q, dtype=np.float32))
    W0 = np.ascontiguousarray(np.asarray(W0, dtype=np.float32))

    in_maps = [
        {
            "C": C[i * BPC : (i + 1) * BPC],
            "Q": Q[i * BPC : (i + 1) * BPC],
            "W0": W0,
        }
        for i in range(NCORES)
    ]
    res = run_bass_kernel_spmd(nc, in_maps, core_ids=list(range(NCORES)))
    A = np.concatenate([res.results[i]["A"] for i in range(NCORES)], axis=0)
    Bt = np.concatenate([res.results[i]["Bt"] for i in range(NCORES)], axis=0)
    return (A, Bt)
